# revision 9
# baseline (speedup 1.0000x reference)
"""Trainium2 Bass kernel: out = x @ ((W_int + offset) * scale), fp8 DoubleRow.

Math (same quantization as the 343us baseline): V = W - 63 (|V| <= 63), cast
to fp8 e4m3; x cast to fp8 e4m3. Then
  out[m,n] = scale[n] * ((x8 @ V8)[m,n] + (63 + offset[n]) * rowsum(x)[m])
with the rank-1 term in exact f32 (DVE), so only x/V carry fp8 error.
Measured rel err ~1.845e-2 vs f64 (threshold 2e-2).

v2 changes vs the N-sharded baseline (338-343us):
- Shard along M instead of N: each core owns m-cols [c*512, (c+1)*512) and
  ALL of N. N = 11008 = 86*128 exactly, so there are no padded stationary
  blocks: 86 nb * 16 kp = 1376 DoubleRow MMs per core (vs 1408 padded),
  a 2.3% shorter PE stream. W (45MB fp8) is streamed per-nb (512KB tiles,
  148 GB/s sustained) instead of kept resident.
- Warmup: first W/x DMAs are issued as small pieces spread over 4 queues
  (scalar/sync/vector) so the first real MM starts ~9us instead of ~13us;
  ~16 dummy DoubleRow MMs on memset tiles keep the PE busy from ~6.5us so
  the HAM clock-gate reaches K=8/8 before the real stream begins (baseline
  ran at 1.2GHz until 30us, ~10us penalty).
- Output staged in bf16 (halves out DMA; adds ~0.1% RMS rounding, total err
  budget unaffected), epilogue per nb is [128,512] so the post-last-MM tail
  chain is ~2us instead of ~4us.

Per-core schedule: for nb in 0..85: 16 kp MMs accumulate into one PSUM bank
([128n, 512m], moving x chunk [128,2,512]); epilogue: DVE STT adds
rowsum*(63+off), ACT applies scale -> bf16 SBUF, gpsimd DMA to DRAM
outt[(nb p), m]. Host un-transposes and concatenates the 8 m-slices.
"""

import numpy as np
import ml_dtypes

M, K, N = 4096, 4096, 11008
NCORES = 8
MSH = M // NCORES          # 512 m-cols per core
P = 128
KP = 16                    # k-pairs of 256
NB = N // P                # 86 n-blocks, exact
WBUFS = 8                  # streamed W tiles in flight
NWARM = 34                 # dummy HAM-warmup matmuls (bridge until real data)

_E4 = ml_dtypes.float8_e4m3

_cache = {}


def _build_nc():
    import concourse.bacc as bacc
    import concourse.mybir as mybir
    import concourse.tile as tile

    fp8 = mybir.dt.float8e4
    f32 = mybir.dt.float32
    bf16 = mybir.dt.bfloat16
    DR = mybir.MatmulPerfMode.DoubleRow
    Copy = mybir.ActivationFunctionType.Copy

    nc = bacc.Bacc(None, target_bir_lowering=False)
    # xq rows: kp*P + p ; cols: slot*MSH + m  (x^T fp8 pairs, this core's m)
    xq = nc.dram_tensor("xq", [KP * P, 2 * MSH], fp8, kind="ExternalInput")
    # wq rows: nb*P + p ; cols: kp*256 + slot*128 + nn  (full W, fp8 pairs)
    wq = nc.dram_tensor("wq", [NB * P, KP * 2 * P], fp8, kind="ExternalInput")
    sbc = nc.dram_tensor("sbc", [P, MSH], f32, kind="ExternalInput")
    offc = nc.dram_tensor("offc", [P, NB], f32, kind="ExternalInput")
    scalec = nc.dram_tensor("scalec", [P, NB], f32, kind="ExternalInput")
    outt = nc.dram_tensor("outt", [NB * P, MSH], bf16, kind="ExternalOutput")

    xq3 = xq.ap().rearrange("(kp p) f -> p kp f", p=P)     # [128, 16, 1024]
    wq3 = wq.ap().rearrange("(nb p) f -> p nb f", p=P)     # [128, 86, 4096]
    outt3 = outt.ap().rearrange("(nb p) m -> p nb m", p=P)  # [128, 86, 512]

    with tile.TileContext(nc) as tc:
        with (
            tc.tile_pool(name="wpool", bufs=WBUFS) as wpool,
            tc.tile_pool(name="xpool", bufs=1) as xpool,
            tc.tile_pool(name="cpool", bufs=1) as cpool,
            tc.tile_pool(name="opool", bufs=3) as opool,
            tc.tile_pool(name="psp", bufs=4, space="PSUM") as psp,
            tc.tile_pool(name="pswarm", bufs=1, space="PSUM") as pswarm,
        ):
            # --- HAM warmup: memset junk tiles, dummy MMs keep PE busy ---
            wm = cpool.tile([P, 2, P], fp8, tag="wm")
            xm = cpool.tile([P, 2, P], fp8, tag="xm")
            nc.gpsimd.memset(wm[:], 0)
            nc.gpsimd.memset(xm[:], 0)
            psw = pswarm.tile([P, P], f32, tag="psw")
            for _ in range(NWARM):
                nc.tensor.matmul(
                    psw[:], wm[:], xm[:], start=True, stop=True, perf_mode=DR
                )

            # --- first-wave DMAs: small pieces, issue order = need order ---
            x_sb = []
            for c in range(4):
                x_sb.append(
                    xpool.tile([P, 4, 2, MSH], fp8, tag=f"xc{c}", name=f"x{c}")
                )

            def load_x(c, k0, nk, eng):
                eng.dma_start(
                    x_sb[c][:, k0:k0 + nk, :, :],
                    xq3[:, 4 * c + k0:4 * c + k0 + nk, :].rearrange(
                        "p k (s m) -> p k s m", s=2
                    ),
                )

            w_sb = []
            t0 = wpool.tile([P, KP, 2, P], fp8, tag="w", name="w0")
            t1 = wpool.tile([P, KP, 2, P], fp8, tag="w", name="w1")
            w_sb += [t0, t1]

            def load_w0(q, eng):  # w0 kp-quarter
                eng.dma_start(
                    t0[:, 4 * q:4 * q + 4, :, :],
                    wq3[:, 0, 1024 * q:1024 * (q + 1)].rearrange(
                        "p (k s n) -> p k s n", k=4, s=2
                    ),
                )

            # sync: interleave w0 quarters with x pieces, arrival tracks the
            # cold-MM consumption order (w0.q_i covers kp4i..4i+3).
            load_w0(0, nc.sync)
            load_x(0, 0, 1, nc.sync)    # kp0
            load_x(0, 1, 1, nc.sync)    # kp1
            load_w0(1, nc.sync)
            load_x(0, 2, 2, nc.sync)    # kp2-3
            load_x(1, 0, 2, nc.sync)    # kp4-5
            load_w0(2, nc.sync)
            load_x(1, 2, 2, nc.sync)    # kp6-7
            load_w0(3, nc.sync)
            load_x(2, 0, 2, nc.sync)    # kp8-9
            load_x(2, 2, 2, nc.sync)    # kp10-11

            # gpsimd (free after memsets): the last-consumed x pieces
            load_x(3, 0, 2, nc.gpsimd)  # kp12-13
            load_x(3, 2, 2, nc.gpsimd)  # kp14-15

            # scalar (after its ACT table load): w1 halves + constants
            for h in range(2):
                nc.scalar.dma_start(
                    t1[:, 8 * h:8 * h + 8, :, :],
                    wq3[:, 1, 2048 * h:2048 * (h + 1)].rearrange(
                        "p (k s n) -> p k s n", k=8, s=2
                    ),
                )
            sbc_sb = cpool.tile([P, MSH], f32, tag="sbc")
            nc.scalar.dma_start(sbc_sb[:], sbc.ap())
            offc_sb = cpool.tile([P, NB], f32, tag="offc")
            nc.scalar.dma_start(offc_sb[:], offc.ap())
            scalec_sb = cpool.tile([P, NB], f32, tag="scalec")
            nc.scalar.dma_start(scalec_sb[:], scalec.ap())

            # remaining W stream on sync, paced by wpool buf releases
            def load_w(nb):
                t = wpool.tile([P, KP, 2, P], fp8, tag="w", name=f"w{nb}")
                nc.sync.dma_start(
                    t[:],
                    wq3[:, nb, :].rearrange("p (k s n) -> p k s n", k=KP, s=2),
                )
                w_sb.append(t)

            for nb in range(2, WBUFS):
                load_w(nb)

            # --- main loop: 86 nb groups; the last one in two m-halves so
            # its epilogue overlaps the final MMs ---
            def epilogue(nb, ps, m0, m1):
                # ps += (63 + offset[n]) * rowsum_x[m]
                nc.vector.scalar_tensor_tensor(
                    ps[:, m0:m1],
                    sbc_sb[:, m0:m1],
                    offc_sb[:, nb:nb + 1],
                    ps[:, m0:m1],
                    mybir.AluOpType.mult,
                    mybir.AluOpType.add,
                )
                o_sb = opool.tile([P, m1 - m0], bf16, tag="o")
                nc.scalar.activation(
                    o_sb[:], ps[:, m0:m1], Copy,
                    scale=scalec_sb[:, nb:nb + 1],
                )
                nc.scalar.dma_start(outt3[:, nb, m0:m1], o_sb[:])

            for nb in range(NB):
                if nb + WBUFS < NB:
                    load_w(nb + WBUFS)
                w = w_sb[nb]
                ps = psp.tile([P, MSH], f32, tag="ps")
                halves = ((0, MSH),) if nb < NB - 1 else ((0, 256), (256, MSH))
                for m0, m1 in halves:
                    for kp in range(KP):
                        nc.tensor.matmul(
                            ps[:, m0:m1],
                            w[:, kp, :, :],
                            x_sb[kp // 4][:, kp % 4, :, m0:m1],
                            start=(kp == 0),
                            stop=(kp == KP - 1),
                            perf_mode=DR,
                        )
                    epilogue(nb, ps, m0, m1)
    nc.compile()
    return nc


def _get_nc():
    if "nc" not in _cache:
        _cache["nc"] = _build_nc()
    return _cache["nc"]


def _prep_inputs(x, weight, antiquant_scale, antiquant_offset):
    x = np.asarray(x, dtype=np.float32)
    weight = np.asarray(weight)
    scale = np.asarray(antiquant_scale, dtype=np.float32)
    off = np.asarray(antiquant_offset, dtype=np.float32)

    xt8 = np.ascontiguousarray(x.astype(_E4).T)         # [K, M] fp8
    rs = x.astype(np.float64).sum(axis=1).astype(np.float32)

    V8 = (weight.astype(np.float32) - 63.0).astype(_E4)  # [K, N]
    # rows (nb, p), cols (kp, slot, nn); k = kp*256 + slot*128 + p
    wdr = np.ascontiguousarray(
        V8.reshape(KP, 2, P, NB, P).transpose(3, 2, 0, 1, 4)
    ).reshape(NB * P, KP * 2 * P)
    offc = np.ascontiguousarray((63.0 + off).reshape(NB, P).T)
    scalec = np.ascontiguousarray(scale.reshape(NB, P).T)

    in_maps = []
    for c in range(NCORES):
        sl = slice(c * MSH, (c + 1) * MSH)
        xdr = np.ascontiguousarray(
            xt8[:, sl].reshape(KP, 2, P, MSH).transpose(0, 2, 1, 3)
        ).reshape(KP * P, 2 * MSH)
        sbc = np.ascontiguousarray(
            np.broadcast_to(rs[sl][None, :], (P, MSH))
        )
        in_maps.append({
            "xq": xdr,
            "wq": wdr,
            "sbc": sbc,
            "offc": offc,
            "scalec": scalec,
        })
    return in_maps


def kernel(x, weight, antiquant_scale, antiquant_offset, _trace=False):
    from concourse.bass_utils import run_bass_kernel_spmd

    nc = _get_nc()
    in_maps = _prep_inputs(x, weight, antiquant_scale, antiquant_offset)
    res = run_bass_kernel_spmd(
        nc, in_maps, core_ids=list(range(NCORES)), trace=_trace
    )
    out = np.empty((M, N), dtype=np.float32)
    for c in range(NCORES):
        outt = np.asarray(res.results[c]["outt"])      # [N, MSH] bf16
        out[c * MSH:(c + 1) * MSH, :] = outt.T.astype(np.float32)
    if _trace:
        _cache["last_result"] = res
    return out


# revision 12
# speedup vs baseline: 1.0116x; 1.0116x over previous
"""Trainium2 Bass kernel: out = x @ ((W_int + offset) * scale), fp8 DoubleRow.

Math (same quantization as the 343us baseline): V = W - 63 (|V| <= 63), cast
to fp8 e4m3; x cast to fp8 e4m3. Then
  out[m,n] = scale[n] * ((x8 @ V8)[m,n] + (63 + offset[n]) * rowsum(x)[m])
with the rank-1 term in exact f32 (DVE), so only x/V carry fp8 error.
Measured rel err ~1.845e-2 vs f64 (threshold 2e-2).

v2 changes vs the N-sharded baseline (338-343us):
- Shard along M instead of N: each core owns m-cols [c*512, (c+1)*512) and
  ALL of N. N = 11008 = 86*128 exactly, so there are no padded stationary
  blocks: 86 nb * 16 kp = 1376 DoubleRow MMs per core (vs 1408 padded),
  a 2.3% shorter PE stream. W (45MB fp8) is streamed per-nb (512KB tiles,
  148 GB/s sustained) instead of kept resident.
- Warmup: first W/x DMAs are issued as small pieces spread over 4 queues
  (scalar/sync/vector) so the first real MM starts ~9us instead of ~13us;
  ~16 dummy DoubleRow MMs on memset tiles keep the PE busy from ~6.5us so
  the HAM clock-gate reaches K=8/8 before the real stream begins (baseline
  ran at 1.2GHz until 30us, ~10us penalty).
- Output staged in bf16 (halves out DMA; adds ~0.1% RMS rounding, total err
  budget unaffected), epilogue per nb is [128,512] so the post-last-MM tail
  chain is ~2us instead of ~4us.

Per-core schedule: for nb in 0..85: 16 kp MMs accumulate into one PSUM bank
([128n, 512m], moving x chunk [128,2,512]); epilogue: DVE STT adds
rowsum*(63+off), ACT applies scale -> bf16 SBUF, gpsimd DMA to DRAM
outt[(nb p), m]. Host un-transposes and concatenates the 8 m-slices.
"""

import numpy as np
import ml_dtypes

M, K, N = 4096, 4096, 11008
NCORES = 8
MSH = M // NCORES          # 512 m-cols per core
P = 128
KP = 16                    # k-pairs of 256
NB = N // P                # 86 n-blocks, exact
WBUFS = 8                  # streamed W tiles in flight
NWARM = 20                 # dummy HAM-warmup matmuls (bridge until real data)

_E4 = ml_dtypes.float8_e4m3

_cache = {}


def _build_nc():
    import concourse.bacc as bacc
    import concourse.mybir as mybir
    import concourse.tile as tile

    fp8 = mybir.dt.float8e4
    f32 = mybir.dt.float32
    bf16 = mybir.dt.bfloat16
    DR = mybir.MatmulPerfMode.DoubleRow
    Copy = mybir.ActivationFunctionType.Copy

    nc = bacc.Bacc(None, target_bir_lowering=False)
    # xq rows: kp*P + p ; cols: slot*MSH + m  (x^T fp8 pairs, this core's m)
    xq = nc.dram_tensor("xq", [KP * P, 2 * MSH], fp8, kind="ExternalInput")
    # wq rows: nb*P + p ; cols: kp*256 + slot*128 + nn  (full W, fp8 pairs)
    wq = nc.dram_tensor("wq", [NB * P, KP * 2 * P], fp8, kind="ExternalInput")
    sbc = nc.dram_tensor("sbc", [P, MSH], f32, kind="ExternalInput")
    offc = nc.dram_tensor("offc", [P, NB], f32, kind="ExternalInput")
    scalec = nc.dram_tensor("scalec", [P, NB], f32, kind="ExternalInput")
    outt = nc.dram_tensor("outt", [NB * P, MSH], bf16, kind="ExternalOutput")

    xq3 = xq.ap().rearrange("(kp p) f -> p kp f", p=P)     # [128, 16, 1024]
    wq3 = wq.ap().rearrange("(nb p) f -> p nb f", p=P)     # [128, 86, 4096]
    outt3 = outt.ap().rearrange("(nb p) m -> p nb m", p=P)  # [128, 86, 512]

    with tile.TileContext(nc) as tc:
        with (
            tc.tile_pool(name="wpool", bufs=WBUFS) as wpool,
            tc.tile_pool(name="xpool", bufs=1) as xpool,
            tc.tile_pool(name="cpool", bufs=1) as cpool,
            tc.tile_pool(name="opool", bufs=3) as opool,
            tc.tile_pool(name="psp", bufs=4, space="PSUM") as psp,
            tc.tile_pool(name="pswarm", bufs=1, space="PSUM") as pswarm,
        ):
            # --- HAM warmup: memset junk tiles, dummy MMs keep PE busy ---
            wm = cpool.tile([P, 2, P], fp8, tag="wm")
            xm = cpool.tile([P, 2, P], fp8, tag="xm")
            nc.gpsimd.memset(wm[:], 0)
            nc.gpsimd.memset(xm[:], 0)
            psw = pswarm.tile([P, P], f32, tag="psw")
            for _ in range(NWARM):
                nc.tensor.matmul(
                    psw[:], wm[:], xm[:], start=True, stop=True, perf_mode=DR
                )

            # --- first-wave DMAs: small pieces, issue order = need order ---
            x_sb = []
            for c in range(4):
                x_sb.append(
                    xpool.tile([P, 4, 2, MSH], fp8, tag=f"xc{c}", name=f"x{c}")
                )

            def load_x(c, k0, nk, eng):
                eng.dma_start(
                    x_sb[c][:, k0:k0 + nk, :, :],
                    xq3[:, 4 * c + k0:4 * c + k0 + nk, :].rearrange(
                        "p k (s m) -> p k s m", s=2
                    ),
                )

            w_sb = []
            t0 = wpool.tile([P, KP, 2, P], fp8, tag="w", name="w0")
            t1 = wpool.tile([P, KP, 2, P], fp8, tag="w", name="w1")
            w_sb += [t0, t1]

            def load_w0(q, eng):  # w0 kp-quarter
                eng.dma_start(
                    t0[:, 4 * q:4 * q + 4, :, :],
                    wq3[:, 0, 1024 * q:1024 * (q + 1)].rearrange(
                        "p (k s n) -> p k s n", k=4, s=2
                    ),
                )

            # sync: interleave w0 quarters with x pieces, arrival tracks the
            # cold-MM consumption order (w0.q_i covers kp4i..4i+3).
            load_w0(0, nc.sync)
            load_x(0, 0, 1, nc.sync)    # kp0
            load_x(0, 1, 1, nc.sync)    # kp1
            load_w0(1, nc.sync)
            load_x(0, 2, 2, nc.sync)    # kp2-3
            load_x(1, 0, 2, nc.sync)    # kp4-5
            load_w0(2, nc.sync)
            load_x(1, 2, 2, nc.sync)    # kp6-7
            load_w0(3, nc.sync)
            load_x(2, 0, 2, nc.sync)    # kp8-9
            load_x(2, 2, 2, nc.sync)    # kp10-11

            # gpsimd (free after memsets): the last-consumed x pieces
            load_x(3, 0, 2, nc.gpsimd)  # kp12-13
            load_x(3, 2, 2, nc.gpsimd)  # kp14-15

            # scalar (after its ACT table load): w1 halves + constants
            for h in range(2):
                nc.scalar.dma_start(
                    t1[:, 8 * h:8 * h + 8, :, :],
                    wq3[:, 1, 2048 * h:2048 * (h + 1)].rearrange(
                        "p (k s n) -> p k s n", k=8, s=2
                    ),
                )
            sbc_sb = cpool.tile([P, MSH], f32, tag="sbc")
            nc.scalar.dma_start(sbc_sb[:], sbc.ap())
            offc_sb = cpool.tile([P, NB], f32, tag="offc")
            nc.scalar.dma_start(offc_sb[:], offc.ap())
            scalec_sb = cpool.tile([P, NB], f32, tag="scalec")
            nc.scalar.dma_start(scalec_sb[:], scalec.ap())

            # remaining W stream on sync, paced by wpool buf releases
            def load_w(nb):
                t = wpool.tile([P, KP, 2, P], fp8, tag="w", name=f"w{nb}")
                nc.sync.dma_start(
                    t[:],
                    wq3[:, nb, :].rearrange("p (k s n) -> p k s n", k=KP, s=2),
                )
                w_sb.append(t)

            for nb in range(2, WBUFS):
                load_w(nb)

            # --- main loop: 86 nb groups; the last one in two m-halves so
            # its epilogue overlaps the final MMs ---
            def epilogue(nb, ps, m0, m1):
                # ps += (63 + offset[n]) * rowsum_x[m]
                nc.vector.scalar_tensor_tensor(
                    ps[:],
                    sbc_sb[:, m0:m1],
                    offc_sb[:, nb:nb + 1],
                    ps[:],
                    mybir.AluOpType.mult,
                    mybir.AluOpType.add,
                )
                o_sb = opool.tile([P, m1 - m0], bf16, tag="o")
                nc.scalar.activation(
                    o_sb[:], ps[:], Copy,
                    scale=scalec_sb[:, nb:nb + 1],
                )
                nc.scalar.dma_start(outt3[:, nb, m0:m1], o_sb[:])

            for nb in range(NB):
                if nb + WBUFS < NB:
                    load_w(nb + WBUFS)
                w = w_sb[nb]
                last = nb == NB - 1
                # last group: two m-halves in SEPARATE PSUM banks so the
                # second half's MMs overlap the first half's epilogue
                halves = ((0, MSH),) if not last else ((0, 256), (256, MSH))
                for m0, m1 in halves:
                    ps = psp.tile([P, m1 - m0], f32, tag="ps")
                    for kp in range(KP):
                        nc.tensor.matmul(
                            ps[:],
                            w[:, kp, :, :],
                            x_sb[kp // 4][:, kp % 4, :, m0:m1],
                            start=(kp == 0),
                            stop=(kp == KP - 1),
                            perf_mode=DR,
                        )
                    epilogue(nb, ps, m0, m1)
    nc.compile()
    return nc


def _get_nc():
    if "nc" not in _cache:
        _cache["nc"] = _build_nc()
    return _cache["nc"]


def _prep_inputs(x, weight, antiquant_scale, antiquant_offset):
    x = np.asarray(x, dtype=np.float32)
    weight = np.asarray(weight)
    scale = np.asarray(antiquant_scale, dtype=np.float32)
    off = np.asarray(antiquant_offset, dtype=np.float32)

    xt8 = np.ascontiguousarray(x.astype(_E4).T)         # [K, M] fp8
    rs = x.astype(np.float64).sum(axis=1).astype(np.float32)

    V8 = (weight.astype(np.float32) - 63.0).astype(_E4)  # [K, N]
    # rows (nb, p), cols (kp, slot, nn); k = kp*256 + slot*128 + p
    wdr = np.ascontiguousarray(
        V8.reshape(KP, 2, P, NB, P).transpose(3, 2, 0, 1, 4)
    ).reshape(NB * P, KP * 2 * P)
    offc = np.ascontiguousarray((63.0 + off).reshape(NB, P).T)
    scalec = np.ascontiguousarray(scale.reshape(NB, P).T)

    in_maps = []
    for c in range(NCORES):
        sl = slice(c * MSH, (c + 1) * MSH)
        xdr = np.ascontiguousarray(
            xt8[:, sl].reshape(KP, 2, P, MSH).transpose(0, 2, 1, 3)
        ).reshape(KP * P, 2 * MSH)
        sbc = np.ascontiguousarray(
            np.broadcast_to(rs[sl][None, :], (P, MSH))
        )
        in_maps.append({
            "xq": xdr,
            "wq": wdr,
            "sbc": sbc,
            "offc": offc,
            "scalec": scalec,
        })
    return in_maps


def kernel(x, weight, antiquant_scale, antiquant_offset, _trace=False):
    from concourse.bass_utils import run_bass_kernel_spmd

    nc = _get_nc()
    in_maps = _prep_inputs(x, weight, antiquant_scale, antiquant_offset)
    res = run_bass_kernel_spmd(
        nc, in_maps, core_ids=list(range(NCORES)), trace=_trace
    )
    out = np.empty((M, N), dtype=np.float32)
    for c in range(NCORES):
        outt = np.asarray(res.results[c]["outt"])      # [N, MSH] bf16
        out[c * MSH:(c + 1) * MSH, :] = outt.T.astype(np.float32)
    if _trace:
        _cache["last_result"] = res
    return out
